# revision 20
# baseline (speedup 1.0000x reference)
"""Trainium2 Bass kernel for nn_MAB_65068754534455 (dense transformer MAB block).

Computation (per reference):
  q = query @ Wq.T + bq ; k = kv @ Wk.T + bk ; v = kv @ Wv.T + bv
  per head: A = softmax(q k^T / sqrt(hd)) ; o = A v
  x = qheads + o (merged) ; out = x + relu(x @ Wo.T + bo)

Sharding: 8 cores = 4 batches x 2 query-halves (data parallel, no collectives).

v2 design (ACT-bound software pipeline):
- The softmax exp (16.8M elements/core on the Scalar engine) is the hard
  floor (~130us); everything else is arranged to hide under it.
- Attention matmuls run in fp8e4m3 with MatmulPerfMode.DoubleRow
  (0.5 cycles/row): PV pairs two kv-tiles per instruction; scores pair the
  real K tile with a zeroed twin (zero-padded DoubleRow) since the
  contraction (hd=64) can't pair.
- exp is fused with scale=1/8 and bias=-2 (softmax-invariant, keeps fp8
  range: exp(s-2) <= ~30 << 448), output directly in fp8.
- V bias is folded into the residual-q bias (softmax weights sum to 1):
  x = (Qproj + bq + bv) + o_nobias/r.
- Input DMA is sliced in dependency order (d-tile-0 weights + first x
  slabs first) so the PE starts ~8us in; K/Q/V projections and the output
  projection are emitted as PE filler interleaved into the attention
  stream (PE is in-order, so emission order = execution order).
- Softmax normalize: r (ones-column row of the PV accumulator) ->
  reciprocal_approx_fast (DVE) -> partition_broadcast (GpSimd) ->
  mul (DVE) -> add into qt (GpSimd); odd heads shift partitions via a
  small SBUF-to-SBUF DMA.
"""

import math

import numpy as np

import concourse.mybir as mybir
import concourse.tile as tile
from concourse import bacc
from concourse.bass_utils import run_bass_kernel_spmd

# problem constants (hardcoded per spec)
B, SQ, SKV, D, H = 4, 2048, 2048, 512, 8
HD = D // H                      # 64
SCALE = 1.0 / math.sqrt(HD)
NCORES = 8
TQ = SQ // 2                     # 1024 query rows per core

F32 = mybir.dt.float32
F32R = mybir.dt.float32r
BF16 = mybir.dt.bfloat16
F8 = mybir.dt.float8e4
DR = mybir.MatmulPerfMode.DoubleRow

KT = D // 128                    # 4 contraction k-tiles
DT = D // 128                    # 4 d-tiles (= head pairs)
NQB = TQ // 512                  # 2 query blocks of 512
NKB = SKV // 512                 # 4 kv blocks of 512
NTK = SKV // 128                 # 16 kv tiles of 128
NIT = NTK // 2                   # 8 kv double-tiles
VW = HD + 2                      # 66: V head block: 64 v + ones col + zero pad (even width/alignment for dual-fp8 ldweights)

# head order: odd head of each pair first so the last head per qb is even
# (even heads normalize without the partition-shift DMA in the tail)
HEAD_ORDER = [1, 0, 3, 2, 5, 4, 7, 6]


def _build(debug=False):
    nc = bacc.Bacc(None, target_bir_lowering=False, debug=False)

    xqt = nc.dram_tensor("xqt", [D, TQ], F32R, kind="ExternalInput").ap()
    xkvt = nc.dram_tensor("xkvt", [D, SKV], F32R, kind="ExternalInput").ap()
    wqt = nc.dram_tensor("wqt", [D, D], F32R, kind="ExternalInput").ap()
    wkt = nc.dram_tensor("wkt", [D, D], F32R, kind="ExternalInput").ap()
    wvt = nc.dram_tensor("wvt", [D, D], F32R, kind="ExternalInput").ap()
    wot = nc.dram_tensor("wot", [D, D], F32R, kind="ExternalInput").ap()
    bq4 = nc.dram_tensor("bq4", [128, DT], F32, kind="ExternalInput").ap()
    bk4 = nc.dram_tensor("bk4", [128, DT], F32, kind="ExternalInput").ap()
    bo4 = nc.dram_tensor("bo4", [128, DT], F32, kind="ExternalInput").ap()
    bqv4 = nc.dram_tensor("bqv4", [128, DT], F32, kind="ExternalInput").ap()
    outt = nc.dram_tensor("outt", [D, TQ], F32, kind="ExternalOutput").ap()
    if debug:
        kt_dbg = nc.dram_tensor("kt_dbg", [128, DT * SKV], mybir.dt.bfloat16, kind="ExternalOutput").ap()
        qtb_dbg = nc.dram_tensor("qtb_dbg", [128, DT * TQ], mybir.dt.bfloat16, kind="ExternalOutput").ap()
        v_dbg = nc.dram_tensor("v_dbg", [128, NTK * H * VW], mybir.dt.uint8, kind="ExternalOutput").ap()
        qt_dbg = nc.dram_tensor("qt_dbg", [128, DT * TQ], F32, kind="ExternalOutput").ap()
        r_dbg = nc.dram_tensor("r_dbg", [16, 512], F32, kind="ExternalOutput").ap()
        o_dbg = nc.dram_tensor("o_dbg", [VW, 512], F32, kind="ExternalOutput").ap()
        e_dbg = nc.dram_tensor("e_dbg", [128, 1024], mybir.dt.uint8, kind="ExternalOutput").ap()
        wk_dbg = nc.dram_tensor("wk_dbg", [128, KT * D], F32, kind="ExternalOutput").ap()
        ps_dbg = nc.dram_tensor("ps_dbg", [128, 2, 512], F32, kind="ExternalOutput").ap()

    wq_r = wqt.rearrange("(o p) d -> p o d", p=128)
    wk_r = wkt.rearrange("(o p) d -> p o d", p=128)
    wv_r = wvt.rearrange("(o p) d -> p o d", p=128)
    wo_r = wot.rearrange("(o p) d -> p o d", p=128)
    xq_r = xqt.rearrange("(o p) t -> p o t", p=128)
    xkv_r = xkvt.rearrange("(o p) t -> p o t", p=128)

    with tile.TileContext(nc) as tc:
        with (
            tc.tile_pool(name="persist", bufs=1) as pp,
            tc.tile_pool(name="e2", bufs=4) as ep,
            tc.tile_pool(name="rb", bufs=2) as rbp,
            tc.tile_pool(name="rtmp", bufs=2) as rtp,
            tc.tile_pool(name="rcol", bufs=2) as rcp,
            tc.tile_pool(name="rinv", bufs=2) as rip,
            tc.tile_pool(name="rb1", bufs=2) as rb1p,
            tc.tile_pool(name="on", bufs=2) as onp,
            tc.tile_pool(name="on2", bufs=2) as on2p,
            tc.tile_pool(name="yt", bufs=2) as yp,
            tc.tile_pool(name="pj", bufs=2, space="PSUM") as pjp,
            tc.tile_pool(name="s2", bufs=2, space="PSUM") as sp,
            tc.tile_pool(name="ops", bufs=2, space="PSUM") as opl,
        ):
            w_q = pp.tile([128, KT, D], F32R)
            w_k = pp.tile([128, KT, D], F32R)
            w_v = pp.tile([128, KT, D], F32R)
            w_o = pp.tile([128, KT, D], F32R)
            qt = pp.tile([128, DT, TQ], F32R)          # Qproj + bq + bv (x base)
            qtb = pp.tile([128, DT, TQ], BF16)         # Qproj + bq (scores)
            kt = pp.tile([128, DT, SKV], BF16)         # K^T + bk (scores lhsT)
            v = pp.tile([128, NTK, H, VW], F8)         # col 64 of each head = 1
            nb2 = pp.tile([128, 1], F32)
            bq_s = pp.tile([128, DT], F32)
            bk_s = pp.tile([128, DT], F32)
            bo_s = pp.tile([128, DT], F32)
            bqv_s = pp.tile([128, DT], F32)

            # constant regions (zero pads / ones columns)
            nc.vector.memset(nb2[:], -2.0)
            nc.vector.memset(v[:, :, :, HD], 1.0)
            nc.vector.memset(v[:, :, :, HD + 1], 0.0)

            # input DMAs, sliced in dependency order (first-needed first)
            nc.sync.dma_start(bq_s[:], bq4[:])
            nc.sync.dma_start(bk_s[:], bk4[:])
            nc.sync.dma_start(bo_s[:], bo4[:])
            nc.sync.dma_start(bqv_s[:], bqv4[:])
            xq_s = pp.tile([128, KT, TQ], F32R)
            xkv_s = pp.tile([128, KT, SKV], F32R)
            nc.sync.dma_start(w_k[:], wk_r)
            for b in range(NKB):
                nc.sync.dma_start(
                    xkv_s[:, :, b * 512 : (b + 1) * 512],
                    xkv_r[:, :, b * 512 : (b + 1) * 512],
                )
            nc.sync.dma_start(w_v[:], wv_r)
            nc.sync.dma_start(w_q[:], wq_r)
            nc.sync.dma_start(xq_s[:], xq_r)
            nc.sync.dma_start(w_o[:], wo_r)

            # ---------------- emission units ----------------
            def KP(j, b):
                """K^T projection: d-tile j, kv block b -> kt tiles 4b..4b+3."""
                ps = pjp.tile([128, 512], F32, tag="pj", name="pjt")
                for k in range(KT):
                    nc.tensor.matmul(
                        ps[:], w_k[:, k, j * 128 : (j + 1) * 128],
                        xkv_s[:, k, b * 512 : (b + 1) * 512],
                        start=(k == 0), stop=(k == KT - 1),
                    )
                if debug and (j, b) in ((0, 0), (2, 0)):
                    psc = onp.tile([128, 512], F32, name="psdbg")
                    nc.vector.tensor_copy(psc[:], ps[:])
                    nc.sync.dma_start(ps_dbg[:, 0 if j == 0 else 1, :], psc[:])
                nc.vector.tensor_scalar_add(
                    kt[:, j, b * 512 : (b + 1) * 512], ps[:], bk_s[:, j : j + 1]
                )

            def QP(j, qb):
                """Q^T projection: d-tile j, q block qb -> qt (f32) + qtb (fp8)."""
                ps = pjp.tile([128, 512], F32, tag="pj", name="pjt")
                for k in range(KT):
                    nc.tensor.matmul(
                        ps[:], w_q[:, k, j * 128 : (j + 1) * 128],
                        xq_s[:, k, qb * 512 : (qb + 1) * 512],
                        start=(k == 0), stop=(k == KT - 1),
                    )
                qsl = slice(qb * 512, (qb + 1) * 512)
                nc.vector.tensor_scalar_add(qt[:, j, qsl], ps[:], bqv_s[:, j : j + 1])
                nc.vector.tensor_scalar_add(qtb[:, j, qsl], ps[:], bq_s[:, j : j + 1])

            def VP(i):
                """V projection (no bias): kv tile i -> v fp8."""
                ps = pjp.tile([128, 512], F32, tag="pj", name="pjt")
                for k in range(KT):
                    nc.tensor.matmul(
                        ps[:], xkv_s[:, k, i * 128 : (i + 1) * 128], w_v[:, k, :],
                        start=(k == 0), stop=(k == KT - 1),
                    )
                nc.vector.tensor_copy(
                    v[:, i, :, 0:HD],
                    ps[:].rearrange("p (h w) -> p h w", w=HD),
                )

            def A(h, qb, it, o_ps):
                """Attention step: head h, q block qb, kv double-tile it."""
                hp, half = h // 2, h % 2
                prows = slice(64 * half, 64 * half + 64)
                qsl = slice(qb * 512, (qb + 1) * 512)
                s2 = sp.tile([128, 2, 512], F32, tag="s2", name="s2t")
                for t in range(2):
                    ksl = slice((2 * it + t) * 128, (2 * it + t + 1) * 128)
                    nc.tensor.matmul(
                        s2[:, t, :], kt[prows, hp, ksl], qtb[prows, hp, qsl],
                        start=True, stop=True,
                    )
                e2 = ep.tile([128, 2, 512], F8)
                nc.scalar.activation(
                    e2[:], s2[:], mybir.ActivationFunctionType.Exp,
                    bias=nb2[:], scale=SCALE,
                )
                if debug and h == 5 and qb == 0 and it == 3:
                    nc.sync.dma_start(e_dbg[:, :], e2[:].bitcast(mybir.dt.uint8))
                nc.tensor.matmul(
                    o_ps[0:VW, :], v[:, 2 * it : 2 * it + 2, h, :], e2[:],
                    start=(it == 0), stop=(it == NIT - 1), perf_mode=DR,
                )

            def RCOLLECT(half, o_ps, rcol):
                """stash this head's r row (PSUM row 64) into rcol[half]."""
                rtmp = rtp.tile([128, 512], F32)
                nc.vector.tensor_copy(rtmp[HD : HD + 1, :], o_ps[HD : HD + 1, :])
                nc.sync.dma_start(rcol[half : half + 1, :], rtmp[HD : HD + 1, :])

            def NPAIR(hp, qb, pair):
                """normalize + residual-add both heads of d-tile hp."""
                qsl = slice(qb * 512, (qb + 1) * 512)
                rcol = pair["rcol"]
                rinv2 = rip.tile([2, 512], F32)
                nc.vector.reciprocal(rinv2[:], rcol[:])
                for half, o_ps in pair["o"].items():
                    rb1 = rb1p.tile([1, 512], F32)
                    nc.sync.dma_start(rb1[0:1, :], rinv2[half : half + 1, :])
                    rb = rbp.tile([64, 512], F32)
                    nc.gpsimd.partition_broadcast(rb[:], rb1[0:1, :])
                    on = onp.tile([64, 512], F32)
                    nc.vector.tensor_mul(on[:], o_ps[0:HD, :], rb[:])
                    if half == 0:
                        nc.gpsimd.tensor_add(
                            qt[0:HD, hp, qsl], qt[0:HD, hp, qsl], on[:]
                        )
                    else:
                        on2 = on2p.tile([128, 512], F32)
                        nc.sync.dma_start(on2[64:128, :], on[:])
                        nc.gpsimd.tensor_add(
                            qt[64:128, hp, qsl], qt[64:128, hp, qsl], on2[64:128, :]
                        )

            def P3(j, qb):
                """output projection d-tile j, q block qb + relu-residual + store."""
                qsl = slice(qb * 512, (qb + 1) * 512)
                z = pjp.tile([128, 512], F32, tag="pj", name="pjt")
                for k in range(KT):
                    nc.tensor.matmul(
                        z[:], w_o[:, k, j * 128 : (j + 1) * 128], qt[:, k, qsl],
                        start=(k == 0), stop=(k == KT - 1),
                    )
                yt = yp.tile([128, 512], F32)
                nc.vector.tensor_scalar(
                    yt[:], z[:], bo_s[:, j : j + 1], 0.0,
                    mybir.AluOpType.add, mybir.AluOpType.max,
                )
                nc.vector.tensor_add(yt[:], yt[:], qt[:, j, qsl])
                nc.sync.dma_start(outt[j * 128 : (j + 1) * 128, qsl], yt[:])

            # ---------------- phase-separated emission (EXP-B) ----------------
            for b in range(NKB):
                for j in range(DT):
                    KP(j, b)
            for i in range(NTK):
                VP(i)
            for qb in range(NQB):
                for j in range(DT):
                    QP(j, qb)

            for qb in range(NQB):
                for hi, h in enumerate(HEAD_ORDER):
                    hp, half = h // 2, h % 2
                    if hi % 2 == 0:
                        pair = {"rcol": rcp.tile([2, 512], F32, name="rcolt"), "o": {}}
                    o_ps = opl.tile([VW, 512], F32, name="opv")
                    pair["o"][half] = o_ps
                    for it in range(NIT):
                        A(h, qb, it, o_ps)
                    RCOLLECT(half, o_ps, pair["rcol"])
                    if hi % 2 == 1:
                        NPAIR(hp, qb, pair)

            for j in range(DT):
                P3(j, 0)
            for j in range(DT):
                P3(j, 1)

    nc.compile()
    return nc


_NC = None


def _get_nc():
    global _NC
    if _NC is None:
        _NC = _build()
    return _NC


def kernel(**inputs) -> np.ndarray:
    q = np.ascontiguousarray(np.asarray(inputs["query"], dtype=np.float32))
    kv = np.ascontiguousarray(np.asarray(inputs["key_value"], dtype=np.float32))
    bq = np.asarray(inputs["bq"], np.float32)
    bv = np.asarray(inputs["bv"], np.float32)
    shared = {
        "wqt": np.ascontiguousarray(np.asarray(inputs["Wq"], np.float32).T),
        "wkt": np.ascontiguousarray(np.asarray(inputs["Wk"], np.float32).T),
        "wvt": np.ascontiguousarray(np.asarray(inputs["Wv"], np.float32).T),
        "wot": np.ascontiguousarray(np.asarray(inputs["Wo"], np.float32).T),
        "bq4": np.ascontiguousarray(bq.reshape(DT, 128).T),
        "bk4": np.ascontiguousarray(np.asarray(inputs["bk"], np.float32).reshape(DT, 128).T),
        "bo4": np.ascontiguousarray(np.asarray(inputs["bo"], np.float32).reshape(DT, 128).T),
        "bqv4": np.ascontiguousarray((bq + bv).reshape(DT, 128).T),
    }
    in_maps = []
    for c in range(NCORES):
        b, half = divmod(c, 2)
        qs = q[b, half * TQ : (half + 1) * TQ]
        in_maps.append(
            {
                "xqt": np.ascontiguousarray(qs.T),
                "xkvt": np.ascontiguousarray(kv[b].T),
                **shared,
            }
        )

    nc = _get_nc()
    res = run_bass_kernel_spmd(nc, in_maps, core_ids=list(range(NCORES)))
    kernel._last_results = res  # for test harness introspection

    out = np.empty((B, SQ, D), np.float32)
    for c in range(NCORES):
        b, half = divmod(c, 2)
        out[b, half * TQ : (half + 1) * TQ] = res.results[c]["outt"].T
    return out


# revision 21
# speedup vs baseline: 1.3006x; 1.3006x over previous
"""Trainium2 Bass kernel for nn_MAB_65068754534455 (dense transformer MAB block).

Computation (per reference):
  q = query @ Wq.T + bq ; k = kv @ Wk.T + bk ; v = kv @ Wv.T + bv
  per head: A = softmax(q k^T / sqrt(hd)) ; o = A v
  x = qheads + o (merged) ; out = x + relu(x @ Wo.T + bo)

Sharding: 8 cores = 4 batches x 2 query-halves (data parallel, no collectives).

v2 design (ACT-bound software pipeline):
- The softmax exp (16.8M elements/core on the Scalar engine) is the hard
  floor (~130us); everything else is arranged to hide under it.
- Attention matmuls run in fp8e4m3 with MatmulPerfMode.DoubleRow
  (0.5 cycles/row): PV pairs two kv-tiles per instruction; scores pair the
  real K tile with a zeroed twin (zero-padded DoubleRow) since the
  contraction (hd=64) can't pair.
- exp is fused with scale=1/8 and bias=-2 (softmax-invariant, keeps fp8
  range: exp(s-2) <= ~30 << 448), output directly in fp8.
- V bias is folded into the residual-q bias (softmax weights sum to 1):
  x = (Qproj + bq + bv) + o_nobias/r.
- Input DMA is sliced in dependency order (d-tile-0 weights + first x
  slabs first) so the PE starts ~8us in; K/Q/V projections and the output
  projection are emitted as PE filler interleaved into the attention
  stream (PE is in-order, so emission order = execution order).
- Softmax normalize: r (ones-column row of the PV accumulator) ->
  reciprocal_approx_fast (DVE) -> partition_broadcast (GpSimd) ->
  mul (DVE) -> add into qt (GpSimd); odd heads shift partitions via a
  small SBUF-to-SBUF DMA.
"""

import math

import numpy as np

import concourse.mybir as mybir
import concourse.tile as tile
from concourse import bacc
from concourse.bass_utils import run_bass_kernel_spmd

# problem constants (hardcoded per spec)
B, SQ, SKV, D, H = 4, 2048, 2048, 512, 8
HD = D // H                      # 64
SCALE = 1.0 / math.sqrt(HD)
NCORES = 8
TQ = SQ // 2                     # 1024 query rows per core

F32 = mybir.dt.float32
F32R = mybir.dt.float32r
BF16 = mybir.dt.bfloat16
F8 = mybir.dt.float8e4
DR = mybir.MatmulPerfMode.DoubleRow

KT = D // 128                    # 4 contraction k-tiles
DT = D // 128                    # 4 d-tiles (= head pairs)
NQB = TQ // 512                  # 2 query blocks of 512
NKB = SKV // 512                 # 4 kv blocks of 512
NTK = SKV // 128                 # 16 kv tiles of 128
NIT = NTK // 2                   # 8 kv double-tiles
VW = HD + 2                      # 66: V head block: 64 v + ones col + zero pad (even width/alignment for dual-fp8 ldweights)

# head order: odd head of each pair first so the last head per qb is even
# (even heads normalize without the partition-shift DMA in the tail)
HEAD_ORDER = [1, 0, 3, 2, 5, 4, 7, 6]


def _build(debug=False):
    nc = bacc.Bacc(None, target_bir_lowering=False, debug=False)

    xqt = nc.dram_tensor("xqt", [D, TQ], F32R, kind="ExternalInput").ap()
    xkvt = nc.dram_tensor("xkvt", [D, SKV], F32R, kind="ExternalInput").ap()
    wqt = nc.dram_tensor("wqt", [D, D], F32R, kind="ExternalInput").ap()
    wkt = nc.dram_tensor("wkt", [D, D], F32R, kind="ExternalInput").ap()
    wvt = nc.dram_tensor("wvt", [D, D], F32R, kind="ExternalInput").ap()
    wot = nc.dram_tensor("wot", [D, D], F32R, kind="ExternalInput").ap()
    bq4 = nc.dram_tensor("bq4", [128, DT], F32, kind="ExternalInput").ap()
    bk4 = nc.dram_tensor("bk4", [128, DT], F32, kind="ExternalInput").ap()
    bo4 = nc.dram_tensor("bo4", [128, DT], F32, kind="ExternalInput").ap()
    bqv4 = nc.dram_tensor("bqv4", [128, DT], F32, kind="ExternalInput").ap()
    outt = nc.dram_tensor("outt", [D, TQ], F32, kind="ExternalOutput").ap()
    if debug:
        kt_dbg = nc.dram_tensor("kt_dbg", [128, DT * SKV], mybir.dt.bfloat16, kind="ExternalOutput").ap()
        qtb_dbg = nc.dram_tensor("qtb_dbg", [128, DT * TQ], mybir.dt.bfloat16, kind="ExternalOutput").ap()
        v_dbg = nc.dram_tensor("v_dbg", [128, NTK * H * VW], mybir.dt.uint8, kind="ExternalOutput").ap()
        qt_dbg = nc.dram_tensor("qt_dbg", [128, DT * TQ], F32, kind="ExternalOutput").ap()
        r_dbg = nc.dram_tensor("r_dbg", [16, 512], F32, kind="ExternalOutput").ap()
        o_dbg = nc.dram_tensor("o_dbg", [VW, 512], F32, kind="ExternalOutput").ap()
        e_dbg = nc.dram_tensor("e_dbg", [128, 1024], mybir.dt.uint8, kind="ExternalOutput").ap()
        wk_dbg = nc.dram_tensor("wk_dbg", [128, KT * D], F32, kind="ExternalOutput").ap()
        ps_dbg = nc.dram_tensor("ps_dbg", [128, 2, 512], F32, kind="ExternalOutput").ap()

    wq_r = wqt.rearrange("(o p) d -> p o d", p=128)
    wk_r = wkt.rearrange("(o p) d -> p o d", p=128)
    wv_r = wvt.rearrange("(o p) d -> p o d", p=128)
    wo_r = wot.rearrange("(o p) d -> p o d", p=128)
    xq_r = xqt.rearrange("(o p) t -> p o t", p=128)
    xkv_r = xkvt.rearrange("(o p) t -> p o t", p=128)

    with tile.TileContext(nc) as tc:
        with (
            tc.tile_pool(name="persist", bufs=1) as pp,
            tc.tile_pool(name="e2", bufs=4) as ep,
            tc.tile_pool(name="rb", bufs=2) as rbp,
            tc.tile_pool(name="ocf", bufs=4) as ocp,
            tc.tile_pool(name="rcol", bufs=2) as rcp,
            tc.tile_pool(name="rinv", bufs=2) as rip,
            tc.tile_pool(name="rb1", bufs=2) as rb1p,
            tc.tile_pool(name="on", bufs=2) as onp,
            tc.tile_pool(name="on2", bufs=2) as on2p,
            tc.tile_pool(name="yt", bufs=2) as yp,
            tc.tile_pool(name="pj", bufs=2, space="PSUM") as pjp,
            tc.tile_pool(name="s2", bufs=2, space="PSUM") as sp,
            tc.tile_pool(name="ops", bufs=2, space="PSUM") as opl,
        ):
            w_q = pp.tile([128, KT, D], F32R)
            w_k = pp.tile([128, KT, D], F32R)
            w_v = pp.tile([128, KT, D], F32R)
            w_o = pp.tile([128, KT, D], F32R)
            qt = pp.tile([128, DT, TQ], F32R)          # Qproj + bq + bv (x base)
            qtb = pp.tile([128, DT, TQ], BF16)         # Qproj + bq (scores)
            kt = pp.tile([128, DT, SKV], BF16)         # K^T + bk (scores lhsT)
            v = pp.tile([128, NTK, H, VW], F8)         # col 64 of each head = 1
            nb2 = pp.tile([128, 1], F32)
            bq_s = pp.tile([128, DT], F32)
            bk_s = pp.tile([128, DT], F32)
            bo_s = pp.tile([128, DT], F32)
            bqv_s = pp.tile([128, DT], F32)

            # constant regions (zero pads / ones columns)
            nc.vector.memset(nb2[:], -2.0)
            nc.vector.memset(v[:, :, :, HD], 1.0)
            nc.vector.memset(v[:, :, :, HD + 1], 0.0)

            # input DMAs, sliced in dependency order (first-needed first)
            nc.sync.dma_start(bq_s[:], bq4[:])
            nc.sync.dma_start(bk_s[:], bk4[:])
            nc.sync.dma_start(bo_s[:], bo4[:])
            nc.sync.dma_start(bqv_s[:], bqv4[:])
            xq_s = pp.tile([128, KT, TQ], F32R)
            xkv_s = pp.tile([128, KT, SKV], F32R)
            nc.sync.dma_start(w_k[:], wk_r)
            for b in range(NKB):
                nc.sync.dma_start(
                    xkv_s[:, :, b * 512 : (b + 1) * 512],
                    xkv_r[:, :, b * 512 : (b + 1) * 512],
                )
            nc.sync.dma_start(w_v[:], wv_r)
            nc.sync.dma_start(w_q[:], wq_r)
            nc.sync.dma_start(xq_s[:], xq_r)
            nc.sync.dma_start(w_o[:], wo_r)

            # ---------------- emission units ----------------
            def KP(j, b):
                """K^T projection: d-tile j, kv block b -> kt tiles 4b..4b+3."""
                ps = pjp.tile([128, 512], F32, tag="pj", name="pjt")
                for k in range(KT):
                    nc.tensor.matmul(
                        ps[:], w_k[:, k, j * 128 : (j + 1) * 128],
                        xkv_s[:, k, b * 512 : (b + 1) * 512],
                        start=(k == 0), stop=(k == KT - 1),
                    )
                if debug and (j, b) in ((0, 0), (2, 0)):
                    psc = onp.tile([128, 512], F32, name="psdbg")
                    nc.vector.tensor_copy(psc[:], ps[:])
                    nc.sync.dma_start(ps_dbg[:, 0 if j == 0 else 1, :], psc[:])
                nc.vector.tensor_scalar_add(
                    kt[:, j, b * 512 : (b + 1) * 512], ps[:], bk_s[:, j : j + 1]
                )

            def QP(j, qb):
                """Q^T projection: d-tile j, q block qb -> qt (f32) + qtb (fp8)."""
                ps = pjp.tile([128, 512], F32, tag="pj", name="pjt")
                for k in range(KT):
                    nc.tensor.matmul(
                        ps[:], w_q[:, k, j * 128 : (j + 1) * 128],
                        xq_s[:, k, qb * 512 : (qb + 1) * 512],
                        start=(k == 0), stop=(k == KT - 1),
                    )
                qsl = slice(qb * 512, (qb + 1) * 512)
                nc.vector.tensor_scalar_add(qt[:, j, qsl], ps[:], bqv_s[:, j : j + 1])
                nc.vector.tensor_scalar_add(qtb[:, j, qsl], ps[:], bq_s[:, j : j + 1])

            def VP(i):
                """V projection (no bias): kv tile i -> v fp8."""
                ps = pjp.tile([128, 512], F32, tag="pj", name="pjt")
                for k in range(KT):
                    nc.tensor.matmul(
                        ps[:], xkv_s[:, k, i * 128 : (i + 1) * 128], w_v[:, k, :],
                        start=(k == 0), stop=(k == KT - 1),
                    )
                nc.vector.tensor_copy(
                    v[:, i, :, 0:HD],
                    ps[:].rearrange("p (h w) -> p h w", w=HD),
                )

            def A(h, qb, it, o_ps):
                """Attention step: head h, q block qb, kv double-tile it."""
                hp, half = h // 2, h % 2
                prows = slice(64 * half, 64 * half + 64)
                qsl = slice(qb * 512, (qb + 1) * 512)
                s2 = sp.tile([128, 2, 512], F32, tag="s2", name="s2t")
                for t in range(2):
                    ksl = slice((2 * it + t) * 128, (2 * it + t + 1) * 128)
                    nc.tensor.matmul(
                        s2[:, t, :], kt[prows, hp, ksl], qtb[prows, hp, qsl],
                        start=True, stop=True,
                    )
                e2 = ep.tile([128, 2, 512], F8)
                nc.scalar.activation(
                    e2[:], s2[:], mybir.ActivationFunctionType.Exp,
                    bias=nb2[:], scale=SCALE,
                )
                if debug and h == 5 and qb == 0 and it == 3:
                    nc.sync.dma_start(e_dbg[:, :], e2[:].bitcast(mybir.dt.uint8))
                nc.tensor.matmul(
                    o_ps[0:VW, :], v[:, 2 * it : 2 * it + 2, h, :], e2[:],
                    start=(it == 0), stop=(it == NIT - 1), perf_mode=DR,
                )

            def RCOLLECT(half, o_ps, pair):
                """drain o accumulator (o rows + r row) to SBUF, freeing PSUM."""
                ocf = ocp.tile([VW, 512], F32)
                nc.vector.tensor_copy(ocf[:], o_ps[:])
                nc.sync.dma_start(pair["rcol"][half : half + 1, :], ocf[HD : HD + 1, :])
                pair["oc"][half] = ocf

            def NPAIR(hp, qb, pair):
                """normalize + residual-add both heads of d-tile hp."""
                qsl = slice(qb * 512, (qb + 1) * 512)
                rcol = pair["rcol"]
                rinv2 = rip.tile([2, 512], F32)
                nc.vector.reciprocal(rinv2[:], rcol[:])
                for half, ocf in pair["oc"].items():
                    rb1 = rb1p.tile([1, 512], F32)
                    nc.sync.dma_start(rb1[0:1, :], rinv2[half : half + 1, :])
                    rb = rbp.tile([64, 512], F32)
                    nc.gpsimd.partition_broadcast(rb[:], rb1[0:1, :])
                    on = onp.tile([64, 512], F32)
                    nc.vector.tensor_mul(on[:], ocf[0:HD, :], rb[:])
                    if half == 0:
                        nc.gpsimd.tensor_add(
                            qt[0:HD, hp, qsl], qt[0:HD, hp, qsl], on[:]
                        )
                    else:
                        on2 = on2p.tile([128, 512], F32)
                        nc.sync.dma_start(on2[64:128, :], on[:])
                        nc.gpsimd.tensor_add(
                            qt[64:128, hp, qsl], qt[64:128, hp, qsl], on2[64:128, :]
                        )

            def P3(j, qb):
                """output projection d-tile j, q block qb + relu-residual + store."""
                qsl = slice(qb * 512, (qb + 1) * 512)
                z = pjp.tile([128, 512], F32, tag="pj", name="pjt")
                for k in range(KT):
                    nc.tensor.matmul(
                        z[:], w_o[:, k, j * 128 : (j + 1) * 128], qt[:, k, qsl],
                        start=(k == 0), stop=(k == KT - 1),
                    )
                yt = yp.tile([128, 512], F32)
                nc.vector.tensor_scalar(
                    yt[:], z[:], bo_s[:, j : j + 1], 0.0,
                    mybir.AluOpType.add, mybir.AluOpType.max,
                )
                nc.vector.tensor_add(yt[:], yt[:], qt[:, j, qsl])
                nc.sync.dma_start(outt[j * 128 : (j + 1) * 128, qsl], yt[:])

            # ---------------- phase-separated emission (EXP-B) ----------------
            for b in range(NKB):
                for j in range(DT):
                    KP(j, b)
            for i in range(NTK):
                VP(i)
            for qb in range(NQB):
                for j in range(DT):
                    QP(j, qb)

            for qb in range(NQB):
                for hi, h in enumerate(HEAD_ORDER):
                    hp, half = h // 2, h % 2
                    if hi % 2 == 0:
                        pair = {"rcol": rcp.tile([2, 512], F32, name="rcolt"), "oc": {}}
                    o_ps = opl.tile([VW, 512], F32, name="opv")
                    for it in range(NIT):
                        A(h, qb, it, o_ps)
                    RCOLLECT(half, o_ps, pair)
                    if hi % 2 == 1:
                        NPAIR(hp, qb, pair)

            for j in range(DT):
                P3(j, 0)
            for j in range(DT):
                P3(j, 1)

    nc.compile()
    return nc


_NC = None


def _get_nc():
    global _NC
    if _NC is None:
        _NC = _build()
    return _NC


def kernel(**inputs) -> np.ndarray:
    q = np.ascontiguousarray(np.asarray(inputs["query"], dtype=np.float32))
    kv = np.ascontiguousarray(np.asarray(inputs["key_value"], dtype=np.float32))
    bq = np.asarray(inputs["bq"], np.float32)
    bv = np.asarray(inputs["bv"], np.float32)
    shared = {
        "wqt": np.ascontiguousarray(np.asarray(inputs["Wq"], np.float32).T),
        "wkt": np.ascontiguousarray(np.asarray(inputs["Wk"], np.float32).T),
        "wvt": np.ascontiguousarray(np.asarray(inputs["Wv"], np.float32).T),
        "wot": np.ascontiguousarray(np.asarray(inputs["Wo"], np.float32).T),
        "bq4": np.ascontiguousarray(bq.reshape(DT, 128).T),
        "bk4": np.ascontiguousarray(np.asarray(inputs["bk"], np.float32).reshape(DT, 128).T),
        "bo4": np.ascontiguousarray(np.asarray(inputs["bo"], np.float32).reshape(DT, 128).T),
        "bqv4": np.ascontiguousarray((bq + bv).reshape(DT, 128).T),
    }
    in_maps = []
    for c in range(NCORES):
        b, half = divmod(c, 2)
        qs = q[b, half * TQ : (half + 1) * TQ]
        in_maps.append(
            {
                "xqt": np.ascontiguousarray(qs.T),
                "xkvt": np.ascontiguousarray(kv[b].T),
                **shared,
            }
        )

    nc = _get_nc()
    res = run_bass_kernel_spmd(nc, in_maps, core_ids=list(range(NCORES)))
    kernel._last_results = res  # for test harness introspection

    out = np.empty((B, SQ, D), np.float32)
    for c in range(NCORES):
        b, half = divmod(c, 2)
        out[b, half * TQ : (half + 1) * TQ] = res.results[c]["outt"].T
    return out
